# revision 18
# baseline (speedup 1.0000x reference)
"""Exact attention (B=2, N=2048, H=16, D=64, fp32) on 8 Trainium2 NeuronCores.

Sharding: the 32 (batch, head) pairs are split across 8 cores, 4 heads per
core. Each core computes full (non-causal, unscaled) attention for its heads.

Per-core kernel layout (heads processed as 2 head-pairs):
  - Q, K are PE-transposed into [d, n] layout, with head-pair packing: one
    [128, 128] transpose yields head0's d-rows on partitions 0-63 and head1's
    on partitions 64-127.
  - Critical path per (pair, n-half): 16 m-blocks of S^T = K Q^T (two
    concurrent quadrant matmuls, f32r) -> P^T = exp(S^T) on ACT ([128, 1024]
    instructions; ACT is the roofline: N^2 exps at 1 elem/cycle/lane).
    P^T is written as bf16 so a full n-half of P tiles fits in SBUF.
  - O^T[65, n] = V'^T P^T (V' = [V | ones] in bf16; col 64 accumulates the
    softmax denominators) runs LAZILY: each n-half's 64 O-matmuls accumulate
    into 1-bank [65, 512] PSUM chunks and are drained a few per m-block
    during the NEXT n-half's loop, so they never block the S->exp path.
  - Finalize (flush, PE-transpose to [n, 65], reciprocal, scale, DMA out)
    is also drained lazily through a dedicated 1-bank PSUM scratch pool.

PSUM budget (8 banks): S double-buffer 4, O accumulators 2 (1 per head),
transpose scratch 2. The S ring is never borrowed by other work.
"""

import os
import sys

os.environ.setdefault("MYCRO_LOCAL_CACHE", "1")
sys.path.insert(0, "/opt/trn_rl_repo")

import numpy as np

import concourse.bacc as bacc
import concourse.mybir as mybir
import concourse.tile as tile
from concourse.bass_utils import run_bass_kernel_spmd
from concourse.masks import make_identity

f32 = mybir.dt.float32
f32r = mybir.dt.float32r
bf16 = mybir.dt.bfloat16

B, N, H, D = 2, 2048, 16, 64
HEADS_PER_CORE = 4
N_CORES = 8
NH = 1024          # n-half width (S/exp granularity)
N_MB = N // 128    # 16 m-blocks of 128 rows
DV = D + 1         # V plus ones column


ABLATE_EXP = os.environ.get("ABLATE_EXP") == "1"
ABLATE_O = os.environ.get("ABLATE_O") == "1"


def emit_body(nc, q, k, v, out, pools):
    """Emit one full attention pass for 4 heads ([4, N, D] DRAM tensors)."""
    (const, stage, qkt, vstage, vt_p, spool, ppool, opools, tpool,
     otflush, finsb) = pools
    identity = const["identity"]

    lazy: list = []

    def drain(kmax):
        for _ in range(min(kmax, len(lazy))):
            lazy.pop(0)()

    def emit_stage_dmas(pair):
        """Issue staging + V DMAs for a pair; returns (qt, kt, vts, sgs)."""
        h0, h1 = 2 * pair, 2 * pair + 1
        qt = qkt.tile([128, N], f32r, name=f"qt_{pair}", tag="qt")
        kt = qkt.tile([128, N], f32r, name=f"kt_{pair}", tag="kt")
        sgs = {}
        for src, nm in ((q, "q"), (k, "k")):
            # [128, 16, 128] staging: tile t holds rows t*128.. of both heads
            # (h0 in cols 0:64, h1 in 64:128), in 4-tile DMA chunks.
            sg = stage.tile([128, N // 128, 128], f32,
                            name=f"sg_{nm}_{pair}", tag=f"sg_{nm}")
            for g in range(4):
                gt = slice(g * 4, (g + 1) * 4)
                gr = slice(g * 512, (g + 1) * 512)
                nc.sync.dma_start(
                    out=sg[:, gt, 0:64],
                    in_=src[h0, gr, :].rearrange("(t p) d -> p t d", p=128))
                nc.sync.dma_start(
                    out=sg[:, gt, 64:128],
                    in_=src[h1, gr, :].rearrange("(t p) d -> p t d", p=128))
            sgs[nm] = sg
        vts = []
        vss = []
        for hh in (h0, h1):
            vs = vstage.tile([128, N_MB, D], f32, name=f"vs_{hh}", tag=f"vs{hh % 2}")
            nc.sync.dma_start(
                out=vs, in_=v[hh].rearrange("(mb p) d -> p mb d", p=128))
            vt = vt_p.tile([128, N_MB, DV], bf16, name=f"vt_{hh}", tag=f"vt{hh % 2}")
            nc.vector.tensor_copy(vt[:, :, 0:64], vs)          # f32 -> bf16
            nc.vector.tensor_copy(vt[:, :, 64:65], const["ones"])
            vts.append(vt)
            vss.append(vs)
        return qt, kt, vts, sgs, vss

    def transpose_task(pair, sg, dst, nm, t):
        def go():
            tp = tpool.tile([128, 128], f32, name=f"tp_{nm}_{pair}_{t}", tag="tp")
            nc.tensor.transpose(tp, sg[:, t, :], identity)
            # rounding producer: fp32 psum -> f32r sbuf (DVE only: any op
            # queued on the ACT sequencer can head-of-line block exps)
            nc.vector.tensor_copy(dst[:, t * 128:(t + 1) * 128], tp)
        return go

    def alloc_task(pair, nh, i, phase, box):
        def go():
            box.append(opools[i].tile(
                [65, 512], f32, name=f"oacc_{pair}_{nh}_{i}_{phase}",
                tag=f"o{i}"))
        return go

    def mm_task(vts, pts, mb, i, phase, box):
        def go():
            nc.tensor.matmul(
                out=box[0], lhsT=vts[i][:, mb, :], rhs=pts[mb][i][phase],
                start=(mb == 0), stop=(mb == N_MB - 1))
        return go

    def flush_task(pair, nh, hh, i, phase, box):
        def go():
            ots = otflush.tile([65, 512], f32,
                               name=f"ots_{pair}_{nh}_{hh}_{phase}",
                               tag=f"ots{i}")
            nc.vector.tensor_copy(ots, box[0])
            box.append(ots)
        return go

    def fin_task(pair, nh, hh, phase, c, box, ostage_ref):
        def go():
            ots = box[1]
            cc = phase * 4 + c
            fin = tpool.tile([128, 65], f32,
                             name=f"fin_{pair}_{nh}_{hh}_{cc}", tag="tp")
            nc.tensor.transpose(
                fin, ots[:, c * 128:(c + 1) * 128], identity[0:65, 0:65])
            rcp = finsb.tile([128, 1], f32,
                             name=f"rcp_{pair}_{nh}_{hh}_{cc}", tag="rcp")
            nc.vector.reciprocal(rcp, fin[:, 64:65])
            nc.vector.tensor_scalar_mul(
                ostage_ref[0][:, cc, :], fin[:, 0:64], rcp)
            if cc == NH // 128 - 1:
                nc.sync.dma_start(
                    out=out[hh].rearrange("(cc p) d -> p cc d", p=128)[
                        :, nh * (NH // 128):(nh + 1) * (NH // 128), :],
                    in_=ostage_ref[0])
        return go

    # stage pair 0, transpose it (tpool scratch, off the S ring), then issue
    # pair 1's staging DMAs; pair 1's transposes drain lazily during pair 0.
    state = [emit_stage_dmas(0)]
    for t in range(N // 128):
        transpose_task(0, state[0][3]["q"], state[0][0], "q", t)()
        transpose_task(0, state[0][3]["k"], state[0][1], "k", t)()
    state.append(emit_stage_dmas(1))
    for t in range(N // 128):
        lazy.append(transpose_task(1, state[1][3]["q"], state[1][0], "q", t))
        lazy.append(transpose_task(1, state[1][3]["k"], state[1][1], "k", t))

    for pair in range(2):
        qt, kt, vts, sgs, vss = state[pair]
        h0, h1 = 2 * pair, 2 * pair + 1

        for nh in range(N // NH):
            pts = []
            boxes = {(i, 0): [] for i in range(2)}
            # phase-0 accumulators for this n-half (drain before their mms)
            if not ABLATE_O:
                for i in range(2):
                    lazy.append(alloc_task(pair, nh, i, 0, boxes[(i, 0)]))
            for mb in range(N_MB):
                msl = slice(mb * 128, (mb + 1) * 128)
                # 512-wide S/exp chunks: four 1-bank S slots form a ring
                # deep enough that S(mb+1) prefetches under exp(mb). Within
                # each chunk-pair the two heads' matmuls alternate PE
                # row-groups (head0 rows 0-63, head1 rows 64-127) so each
                # weight load overlaps the other quadrant's streaming.
                mbts = [[None, None], [None, None]]
                for j in range(NH // 512):
                    jsl = slice(nh * NH + j * 512, nh * NH + (j + 1) * 512)
                    sps = [spool.tile([128, 512], f32,
                                      name=f"sp_{pair}_{nh}_{mb}_{i}_{j}",
                                      tag="s") for i in range(2)]
                    for i, plo in ((0, 0), (1, 64)):
                        nc.tensor.matmul(
                            out=sps[i], lhsT=kt[plo:plo + 64, msl],
                            rhs=qt[plo:plo + 64, jsl], start=True, stop=True)
                    for i in range(2):
                        pt = ppool.tile([128, 512], bf16,
                                        name=f"pt_{pair}_{nh}_{mb}_{i}_{j}",
                                        tag="p")
                        if ABLATE_EXP:
                            nc.scalar.activation(
                                out=pt[:, 0:8], in_=sps[i][:, 0:8],
                                func=mybir.ActivationFunctionType.Exp)
                        else:
                            nc.scalar.activation(
                                out=pt, in_=sps[i],
                                func=mybir.ActivationFunctionType.Exp)
                        mbts[i][j] = pt
                pts.append(mbts)
                # phase-0 O-matmuls for this mb can run as soon as its P
                # tiles exist; they join the queue right behind.
                if not ABLATE_O:
                    for i in range(2):
                        lazy.append(mm_task(vts, pts, mb, i, 0, boxes[(i, 0)]))
                drain(8)
            # tail block for this n-half: finish phase 0, then all of
            # phase 1 + finalize; drains during the next n-half's loop.
            if ABLATE_O:
                continue
            ostage_refs = {}
            for hh in (h0, h1):
                i = hh % 2
                ostage = finsb.tile([128, NH // 128, 64], f32,
                                    name=f"ostage_{pair}_{nh}_{hh}",
                                    tag=f"ostage{i}")
                ostage_refs[hh] = [ostage]
            for hh in (h0, h1):
                i = hh % 2
                lazy.append(flush_task(pair, nh, hh, i, 0, boxes[(i, 0)]))
            for hh in (h0, h1):
                i = hh % 2
                for c in range(4):
                    lazy.append(fin_task(pair, nh, hh, 0, c, boxes[(i, 0)],
                                         ostage_refs[hh]))
            for i in range(2):
                boxes[(i, 1)] = []
                lazy.append(alloc_task(pair, nh, i, 1, boxes[(i, 1)]))
            for mb in range(N_MB):
                for i in range(2):
                    lazy.append(mm_task(vts, pts, mb, i, 1, boxes[(i, 1)]))
            for hh in (h0, h1):
                i = hh % 2
                lazy.append(flush_task(pair, nh, hh, i, 1, boxes[(i, 1)]))
            for hh in (h0, h1):
                i = hh % 2
                for c in range(4):
                    lazy.append(fin_task(pair, nh, hh, 1, c, boxes[(i, 1)],
                                         ostage_refs[hh]))

    if ABLATE_O:
        # out is otherwise unwritten in this ablation; dump V staging tiles
        for pair in range(2):
            for i, hh in enumerate((2 * pair, 2 * pair + 1)):
                nc.sync.dma_start(
                    out=out[hh].rearrange("(mb p) d -> p mb d", p=128),
                    in_=state[pair][4][i])
    drain(len(lazy))


def make_pools(nc, tc, ctx):
    const_pool = ctx.enter_context(tc.tile_pool(name="const", bufs=1))
    identity = const_pool.tile([128, 128], f32, name="identity")
    make_identity(nc, identity)
    ones = const_pool.tile([128, N_MB, 1], f32, name="ones")
    nc.vector.memset(ones, 1.0)

    stage = ctx.enter_context(tc.tile_pool(name="stage", bufs=2))
    qkt = ctx.enter_context(tc.tile_pool(name="qkt", bufs=2))
    vstage = ctx.enter_context(tc.tile_pool(name="vstage", bufs=2))
    vt_p = ctx.enter_context(tc.tile_pool(name="vt", bufs=2))
    spool = ctx.enter_context(tc.tile_pool(name="spool", bufs=4, space="PSUM"))
    # P tiles must survive a full n-half (their O matmuls drain during
    # the next n-half), so the pool is deep; bf16 keeps it affordable.
    ppool = ctx.enter_context(tc.tile_pool(name="ppool", bufs=72))
    opools = [
        ctx.enter_context(tc.tile_pool(name=f"opool{i}", bufs=1, space="PSUM"))
        for i in range(2)
    ]
    tpool = ctx.enter_context(tc.tile_pool(name="tpool", bufs=2, space="PSUM"))
    otflush = ctx.enter_context(tc.tile_pool(name="otflush", bufs=2))
    finsb = ctx.enter_context(tc.tile_pool(name="finsb", bufs=3))

    return ({"identity": identity, "ones": ones}, stage, qkt, vstage, vt_p,
            spool, ppool, opools, tpool, otflush, finsb)


def build(repeat=1):
    nc = bacc.Bacc("TRN2", target_bir_lowering=False, debug=False)
    q = nc.dram_tensor("q", [HEADS_PER_CORE, N, D], f32, kind="ExternalInput").ap()
    k = nc.dram_tensor("k", [HEADS_PER_CORE, N, D], f32, kind="ExternalInput").ap()
    v = nc.dram_tensor("v", [HEADS_PER_CORE, N, D], f32, kind="ExternalInput").ap()
    out = nc.dram_tensor("out", [HEADS_PER_CORE, N, D], f32, kind="ExternalOutput").ap()

    from contextlib import ExitStack
    with tile.TileContext(nc) as tc, ExitStack() as ctx:
        pools = make_pools(nc, tc, ctx)

        if repeat == 1:
            emit_body(nc, q, k, v, out, pools)
        else:
            # hint_engines: the body far exceeds one IRAM block per engine,
            # so arm the back-edge branch prefetch to avoid ~4us I$-miss
            # stalls per iteration in the timing loop.
            with tc.For_i(0, repeat, 1, hint_engines=(
                    mybir.EngineType.PE, mybir.EngineType.Activation,
                    mybir.EngineType.DVE, mybir.EngineType.SP)):
                emit_body(nc, q, k, v, out, pools)

    nc.compile()
    return nc


_NC_CACHE = {}


def _get_nc(repeat=1):
    if repeat not in _NC_CACHE:
        _NC_CACHE[repeat] = build(repeat)
    return _NC_CACHE[repeat]


def run_sharded(query, key, value, repeat=1, **spmd_kwargs):
    """query/key/value: [B, N, H, D] fp32 -> out [B, H, N, D] fp32."""
    nc = _get_nc(repeat)
    # [B, N, H, D] -> [B*H, N, D]
    qh = np.ascontiguousarray(np.transpose(query, (0, 2, 1, 3))).reshape(B * H, N, D)
    kh = np.ascontiguousarray(np.transpose(key, (0, 2, 1, 3))).reshape(B * H, N, D)
    vh = np.ascontiguousarray(np.transpose(value, (0, 2, 1, 3))).reshape(B * H, N, D)
    in_maps = [
        {
            "q": qh[c * HEADS_PER_CORE:(c + 1) * HEADS_PER_CORE],
            "k": kh[c * HEADS_PER_CORE:(c + 1) * HEADS_PER_CORE],
            "v": vh[c * HEADS_PER_CORE:(c + 1) * HEADS_PER_CORE],
        }
        for c in range(N_CORES)
    ]
    res = run_bass_kernel_spmd(nc, in_maps, core_ids=list(range(N_CORES)),
                               **spmd_kwargs)
    outs = np.stack([res.results[c]["out"] for c in range(N_CORES)])  # [8, 4, N, D]
    return outs.reshape(B, H, N, D)


def kernel(query, key, value):
    query = np.asarray(query, dtype=np.float32)
    key = np.asarray(key, dtype=np.float32)
    value = np.asarray(value, dtype=np.float32)
    return run_sharded(query, key, value)


if __name__ == "__main__":
    rng = np.random.default_rng(0)
    q = rng.standard_normal((B, N, H, D), dtype=np.float32)
    k = rng.standard_normal((B, N, H, D), dtype=np.float32)
    v = rng.standard_normal((B, N, H, D), dtype=np.float32)
    o = kernel(q, k, v)
    print("out shape:", o.shape, o.dtype)


# revision 19
# speedup vs baseline: 1.0640x; 1.0640x over previous
"""Exact attention (B=2, N=2048, H=16, D=64, fp32) on 8 Trainium2 NeuronCores.

Sharding: the 32 (batch, head) pairs are split across 8 cores, 4 heads per
core. Each core computes full (non-causal, unscaled) attention for its heads.

Per-core kernel layout (heads processed as 2 head-pairs):
  - Q, K are PE-transposed into [d, n] layout, with head-pair packing: one
    [128, 128] transpose yields head0's d-rows on partitions 0-63 and head1's
    on partitions 64-127.
  - Critical path per (pair, n-half): 16 m-blocks of S^T = K Q^T (two
    concurrent quadrant matmuls, f32r) -> P^T = exp(S^T) on ACT ([128, 1024]
    instructions; ACT is the roofline: N^2 exps at 1 elem/cycle/lane).
    P^T is written as bf16 so a full n-half of P tiles fits in SBUF.
  - O^T[65, n] = V'^T P^T (V' = [V | ones] in bf16; col 64 accumulates the
    softmax denominators) runs LAZILY: each n-half's 64 O-matmuls accumulate
    into 1-bank [65, 512] PSUM chunks and are drained a few per m-block
    during the NEXT n-half's loop, so they never block the S->exp path.
  - Finalize (flush, PE-transpose to [n, 65], reciprocal, scale, DMA out)
    is also drained lazily through a dedicated 1-bank PSUM scratch pool.

PSUM budget (8 banks): S double-buffer 4, O accumulators 2 (1 per head),
transpose scratch 2. The S ring is never borrowed by other work.
"""

import os
import sys

os.environ.setdefault("MYCRO_LOCAL_CACHE", "1")
sys.path.insert(0, "/opt/trn_rl_repo")

import numpy as np

import concourse.bacc as bacc
import concourse.mybir as mybir
import concourse.tile as tile
from concourse.bass_utils import run_bass_kernel_spmd
from concourse.masks import make_identity

f32 = mybir.dt.float32
f32r = mybir.dt.float32r
bf16 = mybir.dt.bfloat16

B, N, H, D = 2, 2048, 16, 64
HEADS_PER_CORE = 4
N_CORES = 8
NH = 1024          # n-half width (S/exp granularity)
N_MB = N // 128    # 16 m-blocks of 128 rows
DV = D + 1         # V plus ones column


ABLATE_EXP = os.environ.get("ABLATE_EXP") == "1"
ABLATE_O = os.environ.get("ABLATE_O") == "1"


def emit_body(nc, q, k, v, out, pools):
    """Emit one full attention pass for 4 heads ([4, N, D] DRAM tensors)."""
    (const, stage, qkt, vstage, vt_p, spool, ppool, opools, tpool,
     otflush, finsb) = pools
    identity = const["identity"]

    lazy: list = []

    def drain(kmax):
        for _ in range(min(kmax, len(lazy))):
            lazy.pop(0)()

    def emit_stage_dmas(pair):
        """Issue staging + V DMAs for a pair; returns (qt, kt, vts, sgs)."""
        h0, h1 = 2 * pair, 2 * pair + 1
        qt = qkt.tile([128, N], f32r, name=f"qt_{pair}", tag="qt")
        kt = qkt.tile([128, N], f32r, name=f"kt_{pair}", tag="kt")
        sgs = {}
        for src, nm in ((q, "q"), (k, "k")):
            # [128, 16, 128] staging: tile t holds rows t*128.. of both heads
            # (h0 in cols 0:64, h1 in 64:128), in 4-tile DMA chunks.
            sg = stage.tile([128, N // 128, 128], f32,
                            name=f"sg_{nm}_{pair}", tag=f"sg_{nm}")
            for g in range(4):
                gt = slice(g * 4, (g + 1) * 4)
                gr = slice(g * 512, (g + 1) * 512)
                nc.sync.dma_start(
                    out=sg[:, gt, 0:64],
                    in_=src[h0, gr, :].rearrange("(t p) d -> p t d", p=128))
                nc.sync.dma_start(
                    out=sg[:, gt, 64:128],
                    in_=src[h1, gr, :].rearrange("(t p) d -> p t d", p=128))
            sgs[nm] = sg
        vts = []
        vss = []
        for hh in (h0, h1):
            vs = vstage.tile([128, N_MB, D], f32, name=f"vs_{hh}", tag=f"vs{hh % 2}")
            nc.sync.dma_start(
                out=vs, in_=v[hh].rearrange("(mb p) d -> p mb d", p=128))
            vt = vt_p.tile([128, N_MB, DV], bf16, name=f"vt_{hh}", tag=f"vt{hh % 2}")
            nc.vector.tensor_copy(vt[:, :, 0:64], vs)          # f32 -> bf16
            nc.vector.tensor_copy(vt[:, :, 64:65], const["ones"])
            vts.append(vt)
            vss.append(vs)
        return qt, kt, vts, sgs, vss

    def transpose_task(pair, sg, dst, nm, t, pool=None, tag="tp"):
        def go():
            pl = pool if pool is not None else tpool
            tp = pl.tile([128, 128], f32, name=f"tp_{nm}_{pair}_{t}", tag=tag)
            nc.tensor.transpose(tp, sg[:, t, :], identity)
            # rounding producer: fp32 psum -> f32r sbuf (DVE only: any op
            # queued on the ACT sequencer can head-of-line block exps)
            nc.vector.tensor_copy(dst[:, t * 128:(t + 1) * 128], tp)
        return go

    def alloc_task(pair, nh, i, phase, box):
        def go():
            box.append(opools[i].tile(
                [65, 512], f32, name=f"oacc_{pair}_{nh}_{i}_{phase}",
                tag=f"o{i}"))
        return go

    def mm_task(vts, pts, mb, i, phase, box):
        psl = slice(phase * 512, (phase + 1) * 512)

        def go():
            nc.tensor.matmul(
                out=box[0], lhsT=vts[i][:, mb, :], rhs=pts[mb][i][:, psl],
                start=(mb == 0), stop=(mb == N_MB - 1))
        return go

    def flush_task(pair, nh, hh, i, phase, box):
        def go():
            ots = otflush.tile([65, 512], f32,
                               name=f"ots_{pair}_{nh}_{hh}_{phase}",
                               tag=f"ots{i}")
            nc.vector.tensor_copy(ots, box[0])
            box.append(ots)
        return go

    def fin_task(pair, nh, hh, phase, c, box, ostage_ref):
        def go():
            ots = box[1]
            cc = phase * 4 + c
            fin = tpool.tile([128, 65], f32,
                             name=f"fin_{pair}_{nh}_{hh}_{cc}", tag="tp")
            nc.tensor.transpose(
                fin, ots[:, c * 128:(c + 1) * 128], identity[0:65, 0:65])
            rcp = finsb.tile([128, 1], f32,
                             name=f"rcp_{pair}_{nh}_{hh}_{cc}", tag="rcp")
            nc.vector.reciprocal(rcp, fin[:, 64:65])
            nc.vector.tensor_scalar_mul(
                ostage_ref[0][:, cc, :], fin[:, 0:64], rcp)
            if cc == NH // 128 - 1:
                nc.sync.dma_start(
                    out=out[hh].rearrange("(cc p) d -> p cc d", p=128)[
                        :, nh * (NH // 128):(nh + 1) * (NH // 128), :],
                    in_=ostage_ref[0])
        return go

    # stage pair 0, transpose it (tpool scratch, off the S ring), then issue
    # pair 1's staging DMAs; pair 1's transposes drain lazily during pair 0.
    state = [emit_stage_dmas(0)]
    _startpools = [(tpool, "tp"), (opools[0], "o0"), (opools[1], "o1")]
    for t in range(N // 128):
        pl, tg = _startpools[t % 3]
        transpose_task(0, state[0][3]["q"], state[0][0], "q", t, pl, tg)()
        transpose_task(0, state[0][3]["k"], state[0][1], "k", t, pl, tg)()
    state.append(emit_stage_dmas(1))
    for t in range(N // 128):
        lazy.append(transpose_task(1, state[1][3]["q"], state[1][0], "q", t))
        lazy.append(transpose_task(1, state[1][3]["k"], state[1][1], "k", t))

    for pair in range(2):
        qt, kt, vts, sgs, vss = state[pair]
        h0, h1 = 2 * pair, 2 * pair + 1

        for nh in range(N // NH):
            pts = []
            boxes = {(i, 0): [] for i in range(2)}
            # phase-0 accumulators for this n-half (drain before their mms)
            if not ABLATE_O:
                for i in range(2):
                    lazy.append(alloc_task(pair, nh, i, 0, boxes[(i, 0)]))
            for mb in range(N_MB):
                msl = slice(mb * 128, (mb + 1) * 128)
                mbts = []
                sps = [spool.tile([128, NH], f32,
                                  name=f"sp_{pair}_{nh}_{mb}_{i}", tag="s")
                       for i in range(2)]
                # j outer / i inner: consecutive matmuls alternate PE
                # row-groups (head0 rows 0-63, head1 rows 64-127), so each
                # weight load overlaps the other quadrant's streaming and
                # the two heads' matmuls run concurrently in the array.
                for j in range(NH // 512):
                    jsl = slice(nh * NH + j * 512, nh * NH + (j + 1) * 512)
                    osl = slice(j * 512, (j + 1) * 512)
                    for i, plo in ((0, 0), (1, 64)):
                        nc.tensor.matmul(
                            out=sps[i][:, osl], lhsT=kt[plo:plo + 64, msl],
                            rhs=qt[plo:plo + 64, jsl], start=True, stop=True)
                for i in range(2):
                    sp = sps[i]
                    pt = ppool.tile([128, NH], bf16,
                                    name=f"pt_{pair}_{nh}_{mb}_{i}", tag="p")
                    if ABLATE_EXP:
                        nc.scalar.activation(
                            out=pt[:, 0:8], in_=sp[:, 0:8],
                            func=mybir.ActivationFunctionType.Exp)
                    else:
                        nc.scalar.activation(
                            out=pt, in_=sp, func=mybir.ActivationFunctionType.Exp)
                    mbts.append(pt)
                pts.append(mbts)
                # phase-0 O-matmuls for this mb can run as soon as its P
                # tiles exist; they join the queue right behind.
                if not ABLATE_O:
                    for i in range(2):
                        lazy.append(mm_task(vts, pts, mb, i, 0, boxes[(i, 0)]))
                drain(8)
            # tail block for this n-half: finish phase 0, then all of
            # phase 1 + finalize; drains during the next n-half's loop.
            if ABLATE_O:
                continue
            ostage_refs = {}
            for hh in (h0, h1):
                i = hh % 2
                ostage = finsb.tile([128, NH // 128, 64], f32,
                                    name=f"ostage_{pair}_{nh}_{hh}",
                                    tag=f"ostage{i}")
                ostage_refs[hh] = [ostage]
            for hh in (h0, h1):
                i = hh % 2
                lazy.append(flush_task(pair, nh, hh, i, 0, boxes[(i, 0)]))
            for hh in (h0, h1):
                i = hh % 2
                for c in range(4):
                    lazy.append(fin_task(pair, nh, hh, 0, c, boxes[(i, 0)],
                                         ostage_refs[hh]))
            for i in range(2):
                boxes[(i, 1)] = []
                lazy.append(alloc_task(pair, nh, i, 1, boxes[(i, 1)]))
            for mb in range(N_MB):
                for i in range(2):
                    lazy.append(mm_task(vts, pts, mb, i, 1, boxes[(i, 1)]))
            for hh in (h0, h1):
                i = hh % 2
                lazy.append(flush_task(pair, nh, hh, i, 1, boxes[(i, 1)]))
            for hh in (h0, h1):
                i = hh % 2
                for c in range(4):
                    lazy.append(fin_task(pair, nh, hh, 1, c, boxes[(i, 1)],
                                         ostage_refs[hh]))

    if ABLATE_O:
        # out is otherwise unwritten in this ablation; dump V staging tiles
        for pair in range(2):
            for i, hh in enumerate((2 * pair, 2 * pair + 1)):
                nc.sync.dma_start(
                    out=out[hh].rearrange("(mb p) d -> p mb d", p=128),
                    in_=state[pair][4][i])
    drain(len(lazy))


def make_pools(nc, tc, ctx):
    const_pool = ctx.enter_context(tc.tile_pool(name="const", bufs=1))
    identity = const_pool.tile([128, 128], f32, name="identity")
    make_identity(nc, identity)
    ones = const_pool.tile([128, N_MB, 1], f32, name="ones")
    nc.vector.memset(ones, 1.0)

    stage = ctx.enter_context(tc.tile_pool(name="stage", bufs=2))
    qkt = ctx.enter_context(tc.tile_pool(name="qkt", bufs=2))
    vstage = ctx.enter_context(tc.tile_pool(name="vstage", bufs=2))
    vt_p = ctx.enter_context(tc.tile_pool(name="vt", bufs=2))
    spool = ctx.enter_context(tc.tile_pool(name="spool", bufs=2, space="PSUM"))
    # P tiles must survive a full n-half (their O matmuls drain during
    # the next n-half), so the pool is deep; bf16 keeps it affordable.
    ppool = ctx.enter_context(tc.tile_pool(name="ppool", bufs=36))
    opools = [
        ctx.enter_context(tc.tile_pool(name=f"opool{i}", bufs=1, space="PSUM"))
        for i in range(2)
    ]
    tpool = ctx.enter_context(tc.tile_pool(name="tpool", bufs=2, space="PSUM"))
    otflush = ctx.enter_context(tc.tile_pool(name="otflush", bufs=2))
    finsb = ctx.enter_context(tc.tile_pool(name="finsb", bufs=3))

    return ({"identity": identity, "ones": ones}, stage, qkt, vstage, vt_p,
            spool, ppool, opools, tpool, otflush, finsb)


def build(repeat=1):
    nc = bacc.Bacc("TRN2", target_bir_lowering=False, debug=False)
    q = nc.dram_tensor("q", [HEADS_PER_CORE, N, D], f32, kind="ExternalInput").ap()
    k = nc.dram_tensor("k", [HEADS_PER_CORE, N, D], f32, kind="ExternalInput").ap()
    v = nc.dram_tensor("v", [HEADS_PER_CORE, N, D], f32, kind="ExternalInput").ap()
    out = nc.dram_tensor("out", [HEADS_PER_CORE, N, D], f32, kind="ExternalOutput").ap()

    from contextlib import ExitStack
    with tile.TileContext(nc) as tc, ExitStack() as ctx:
        pools = make_pools(nc, tc, ctx)

        if repeat == 1:
            emit_body(nc, q, k, v, out, pools)
        else:
            # hint_engines: the body far exceeds one IRAM block per engine,
            # so arm the back-edge branch prefetch to avoid ~4us I$-miss
            # stalls per iteration in the timing loop.
            with tc.For_i(0, repeat, 1, hint_engines=(
                    mybir.EngineType.PE, mybir.EngineType.Activation,
                    mybir.EngineType.DVE, mybir.EngineType.SP)):
                emit_body(nc, q, k, v, out, pools)

    nc.compile()
    return nc


_NC_CACHE = {}


def _get_nc(repeat=1):
    if repeat not in _NC_CACHE:
        _NC_CACHE[repeat] = build(repeat)
    return _NC_CACHE[repeat]


def run_sharded(query, key, value, repeat=1, **spmd_kwargs):
    """query/key/value: [B, N, H, D] fp32 -> out [B, H, N, D] fp32."""
    nc = _get_nc(repeat)
    # [B, N, H, D] -> [B*H, N, D]
    qh = np.ascontiguousarray(np.transpose(query, (0, 2, 1, 3))).reshape(B * H, N, D)
    kh = np.ascontiguousarray(np.transpose(key, (0, 2, 1, 3))).reshape(B * H, N, D)
    vh = np.ascontiguousarray(np.transpose(value, (0, 2, 1, 3))).reshape(B * H, N, D)
    in_maps = [
        {
            "q": qh[c * HEADS_PER_CORE:(c + 1) * HEADS_PER_CORE],
            "k": kh[c * HEADS_PER_CORE:(c + 1) * HEADS_PER_CORE],
            "v": vh[c * HEADS_PER_CORE:(c + 1) * HEADS_PER_CORE],
        }
        for c in range(N_CORES)
    ]
    res = run_bass_kernel_spmd(nc, in_maps, core_ids=list(range(N_CORES)),
                               **spmd_kwargs)
    outs = np.stack([res.results[c]["out"] for c in range(N_CORES)])  # [8, 4, N, D]
    return outs.reshape(B, H, N, D)


def kernel(query, key, value):
    query = np.asarray(query, dtype=np.float32)
    key = np.asarray(key, dtype=np.float32)
    value = np.asarray(value, dtype=np.float32)
    return run_sharded(query, key, value)


if __name__ == "__main__":
    rng = np.random.default_rng(0)
    q = rng.standard_normal((B, N, H, D), dtype=np.float32)
    k = rng.standard_normal((B, N, H, D), dtype=np.float32)
    v = rng.standard_normal((B, N, H, D), dtype=np.float32)
    o = kernel(q, k, v)
    print("out shape:", o.shape, o.dtype)
